# revision 3
# baseline (speedup 1.0000x reference)
"""Grouped GEMM (MoE expert-parallel) on 8 TRN2 NeuronCores.

Strategy: expert-parallel — core e computes its expert's GEMM as
yT = W_e @ X_e^T, i.e. [2048, 1024] @ [1024, 2048] with the OUTPUT
transposed (features on PSUM partitions, tokens on the free dim); the
host transposes back (free).  bf16 operands + bf16 output storage with
fp32 PSUM accumulation (rel err ~3e-3, well inside the 2e-2 gate)
halve DMA traffic to 16 MB/core so it fully hides under the PE.

PE-side structure: the stationary operand is a weight feature-block
tile [K=128, F=128], reused across 4 consecutive matmuls that stream
all 2048 tokens (4 x 512).  Tile's legalizer emits one LDWEIGHTS per
matmul; dedup_ldweights() removes the redundant repeats (512 -> 128),
saving the serialized weight-load cycles.  x is loaded in 4 token
chunks so the first matmuls start after ~1 MB of DMA, not 4 MB.

Measured (hw-loop differencing, ~1000-GEMM signal): ~138 us/GEMM vs
~160 us for the previous fp32r kernel structure; pure-PE floor of this
shape is ~134 us (sustained 8-core PE clock is ~2.0 GHz, P0 power
state, so 512 N=512-matmuls cannot beat ~131 us).
"""

import numpy as np

import concourse.mybir as mybir
import concourse.tile as tile
from concourse import bacc

NUM_CORES = 8
IN_F = 1024            # K (contraction)
OUT_F = 2048           # N (out features per expert)
CAP = 2048             # token capacity per core (= expected group size)
P = 128
KT = IN_F // P         # 8 k-subtiles
FB = OUT_F // P        # 16 feature blocks (psum partition dim)
TB = CAP // 512        # 4 token blocks (psum free dim)

BF16 = mybir.dt.bfloat16
NP_BF16 = mybir.dt.np(BF16)


def dedup_ldweights(nc):
    """Remove consecutive PE LDWEIGHTS with identical weight APs.

    Tile's legalization inserts one InstLdweights per matmul.  When
    consecutive matmuls in the final PE stream share the same
    stationary operand the repeats are redundant — the array already
    holds the weights.  Only wait/update-free LDWs are removed, and a
    non-matmul PE instruction resets the tracked signature.
    """
    removed = 0
    for f in nc.m.functions:
        for bb in f.blocks:
            insts = bb.instructions
            last_sig = None
            victims = []
            for i in insts:
                if getattr(i, "engine", None) != mybir.EngineType.PE:
                    continue
                if isinstance(i, mybir.InstLdweights):
                    sig = (str(i.ins[0]), str(i.perf_mode),
                           str(i.is_transpose), str(i.tile_position))
                    if (sig == last_sig and not i.has_wait()
                            and not i.has_update()):
                        victims.append(i)
                    else:
                        last_sig = sig
                elif isinstance(i, mybir.InstMatmult):
                    pass  # does not clobber the loaded weights
                else:
                    last_sig = None
            for v in victims:
                insts.remove(v)
            removed += len(victims)
    return removed


def _emit_gemm(nc, xtr, wq, ytr, x_pool, w_pool, y_pool, psum_pool):
    """One grouped-GEMM body: yT[fb*128:(fb+1)*128, :] = W_fb @ xT."""
    x_res = x_pool.tile([P, KT, CAP], BF16, tag="x", name="x_res")
    w_tiles = [
        w_pool.tile([P, KT, P], BF16, tag="w", name=f"w_{fb}")
        for fb in range(FB)
    ]
    # first x chunk + first 2 w tiles first, then the rest: the fb=0
    # matmuls depend only on chunk 0 and w_0.
    nc.sync.dma_start(x_res[:, :, 0:512], xtr[:, :, 0:512])
    for fb in range(2):
        nc.sync.dma_start(
            w_tiles[fb][:], wq[fb].rearrange("p (o f) -> p o f", o=KT))
    for tb in range(1, TB):
        nc.sync.dma_start(x_res[:, :, tb * 512:(tb + 1) * 512],
                          xtr[:, :, tb * 512:(tb + 1) * 512])
    for fb in range(2, FB):
        nc.sync.dma_start(
            w_tiles[fb][:], wq[fb].rearrange("p (o f) -> p o f", o=KT))
    for fb in range(FB):
        psums = [
            psum_pool.tile([P, 512], mybir.dt.float32,
                           name=f"ps_{fb % 2}_{tb}", tag="psum")
            for tb in range(TB)
        ]
        # o-outer order: the stationary w tile is reused across the 4
        # token-block matmuls; dedup_ldweights removes the 3 repeats.
        for o in range(KT):
            for tb in range(TB):
                nc.tensor.matmul(
                    psums[tb],
                    lhsT=w_tiles[fb][:, o, :],
                    rhs=x_res[:, o, tb * 512:(tb + 1) * 512],
                    start=(o == 0),
                    stop=(o == KT - 1),
                )
        y_sb = y_pool.tile([P, CAP], BF16, tag="y")
        for tb in range(TB):
            nc.vector.tensor_copy(
                y_sb[:, tb * 512:(tb + 1) * 512], psums[tb][:])
        nc.sync.dma_start(ytr[:, fb, :], y_sb[:])


def _build(repeat: int = 1, hw_loop: int = 0):
    """Build the per-core Bass program: yT[OUT_F, CAP] = W @ xT.

    xt: [IN_F, CAP]        (X_e^T — K on SBUF partitions)
    wq: [FB, 128, KT*128]  (W_e packed so each feature-block tile is a
                            contiguous 2KB-per-partition DMA)
    yt: [OUT_F, CAP] bf16  (Y_e^T; host transposes back)

    ``repeat`` python-unrolls the body; ``hw_loop`` additionally wraps
    it in a For_i hardware loop (used only by the benchmark).
    """
    nc = bacc.Bacc(None, target_bir_lowering=False, debug=False)
    xt = nc.dram_tensor("xt", [IN_F, CAP], BF16, kind="ExternalInput")
    wq = nc.dram_tensor("wq", [FB, P, KT * P], BF16, kind="ExternalInput")
    yt = nc.dram_tensor("yt", [OUT_F, CAP], BF16, kind="ExternalOutput")
    xtr = xt.rearrange("(o p) m -> p o m", p=P)    # [128, KT, CAP]
    ytr = yt.rearrange("(fb p) m -> p fb m", p=P)  # [128, FB, CAP]

    with tile.TileContext(nc) as tc:
        with (
            tc.tile_pool(name="x_pool", bufs=2) as x_pool,
            tc.tile_pool(name="w_pool", bufs=FB + 2) as w_pool,
            tc.tile_pool(name="y_pool", bufs=3) as y_pool,
            tc.tile_pool(name="psum", bufs=8, space="PSUM") as psum_pool,
        ):
            pools = (x_pool, w_pool, y_pool, psum_pool)
            if hw_loop:
                with tc.For_i(0, hw_loop):
                    for _ in range(repeat):
                        _emit_gemm(nc, xtr, wq, ytr, *pools)
            else:
                for _ in range(repeat):
                    _emit_gemm(nc, xtr, wq, ytr, *pools)
    dedup_ldweights(nc)
    nc.compile()
    return nc


_NC_CACHE: dict = {}


def _get_nc(repeat: int = 1, hw_loop: int = 0):
    key = (repeat, hw_loop)
    if key not in _NC_CACHE:
        _NC_CACHE[key] = _build(repeat, hw_loop)
    return _NC_CACHE[key]


_RUNNER_CACHE: dict = {}


def _get_runner():
    """Jit the 8-core SPMD executable once; reuse across kernel() calls."""
    if "run" in _RUNNER_CACHE:
        return _RUNNER_CACHE["run"]

    import jax
    from jax.sharding import Mesh, PartitionSpec
    from jax.experimental.shard_map import shard_map
    from concourse import bass2jax
    from concourse.bass2jax import _bass_exec_p, install_neuronx_cc_hook

    nc = _get_nc(1)
    install_neuronx_cc_hook()
    assert nc.dbg_addr is None, "rebuild with debug=False"
    partition_name = (
        nc.partition_id_tensor.name if nc.partition_id_tensor else None
    )

    in_names, out_names, out_avals = [], [], []
    for alloc in nc.m.functions[0].allocations:
        if not isinstance(alloc, mybir.MemoryLocationSet):
            continue
        name = alloc.memorylocations[0].name
        if alloc.kind == "ExternalInput":
            if name != partition_name:
                in_names.append(name)
        elif alloc.kind == "ExternalOutput":
            out_names.append(name)
            out_avals.append(
                jax.core.ShapedArray(
                    tuple(alloc.tensor_shape), mybir.dt.np(alloc.dtype)
                )
            )
    n_params = len(in_names)
    all_in_names = list(in_names) + list(out_names)
    if partition_name is not None:
        all_in_names.append(partition_name)
    donate = tuple(range(n_params, n_params + len(out_names)))

    def _body(*args):
        operands = list(args)
        if partition_name is not None:
            operands.append(bass2jax.partition_id_tensor())
        outs = _bass_exec_p.bind(
            *operands,
            out_avals=tuple(out_avals),
            in_names=tuple(all_in_names),
            out_names=tuple(out_names),
            lowering_input_output_aliases=(),
            sim_require_finite=True,
            sim_require_nnan=True,
            nc=nc,
        )
        return tuple(outs)

    devices = jax.devices()[:NUM_CORES]
    mesh = Mesh(np.asarray(devices), ("core",))
    spec = PartitionSpec("core")
    fn = jax.jit(
        shard_map(
            _body, mesh=mesh,
            in_specs=(spec,) * (n_params + len(out_names)),
            out_specs=(spec,) * len(out_names),
            check_rep=False,
        ),
        donate_argnums=donate, keep_unused=True,
    )

    def run(in_maps):
        concat_in = [
            np.concatenate([np.asarray(m[k]) for m in in_maps], axis=0)
            for k in in_names
        ]
        zeros = [
            np.zeros((NUM_CORES * a.shape[0], *a.shape[1:]), a.dtype)
            for a in out_avals
        ]
        outs = fn(*concat_in, *zeros)
        arr = np.asarray(outs[0]).reshape(NUM_CORES, *out_avals[0].shape)
        return [{out_names[0]: arr[c]} for c in range(NUM_CORES)]

    _RUNNER_CACHE["run"] = run
    return run


def _pack_w(w_e):
    """[OUT_F, IN_F] fp32 -> wq [FB, 128, KT*128] bf16 with
    wq[fb, p, o*128+f] = w_e.T[o*128+p, fb*128+f] (contiguous
    2KB-per-partition feature-block DMA tiles)."""
    wT = np.ascontiguousarray(w_e.T)                  # [IN_F, OUT_F]
    wqv = wT.reshape(KT, P, FB, P).transpose(2, 1, 0, 3)
    return np.ascontiguousarray(wqv.reshape(FB, P, KT * P)).astype(NP_BF16)


def _chunk_in_map(x, wq_e, off: int, size: int):
    """Build the per-core input map for one (expert, token-chunk)."""
    xe = np.zeros((CAP, IN_F), np.float32)
    if size > 0:
        xe[:size] = x[off:off + size]
    return {
        "xt": np.ascontiguousarray(xe.T).astype(NP_BF16),
        "wq": wq_e,
    }


def kernel(**inputs) -> np.ndarray:
    x = np.asarray(inputs["input_tokens"], dtype=np.float32)       # [T, K]
    w = np.asarray(inputs["weight_stack"], dtype=np.float32)       # [E, O, K]
    m_sizes = np.asarray(inputs["m_sizes"]).astype(np.int64)       # [E]
    m_offsets = np.asarray(inputs["m_offsets"]).astype(np.int64)   # [E]

    T = x.shape[0]
    E, O, K = w.shape
    assert K == IN_F and O == OUT_F and E == NUM_CORES

    wq_packed = [_pack_w(w[e]) for e in range(E)]

    # Split each expert's contiguous token group into chunks of <= CAP rows
    # (the deterministic setup gives exactly one CAP-sized chunk per expert).
    chunks = []  # (expert, src_off, size)
    for e in range(E):
        off, size = int(m_offsets[e]), int(m_sizes[e])
        off = max(0, min(off, T))
        size = max(0, min(size, T - off))
        pos = 0
        while pos < size:
            c = min(CAP, size - pos)
            chunks.append((e, off + pos, c))
            pos += c

    out = np.zeros((T, O), dtype=np.float32)
    run = _get_runner()
    for batch_start in range(0, len(chunks), NUM_CORES):
        batch = chunks[batch_start:batch_start + NUM_CORES]
        in_maps = [_chunk_in_map(x, wq_packed[e], off, size)
                   for (e, off, size) in batch]
        # SPMD needs a full complement of cores; pad with repeats of map 0.
        while len(in_maps) < NUM_CORES:
            in_maps.append(in_maps[0])
        results = run(in_maps)
        for i, (e, off, size) in enumerate(batch):
            yte = results[i]["yt"]  # [OUT_F, CAP] bf16 (y^T)
            out[off:off + size] += yte[:, :size].T.astype(np.float32)
    return out
